# revision 2
# baseline (speedup 1.0000x reference)
"""Bass/Tile Trainium2 kernel for nn_BaseConchGS (GNN message passing), v3.

Data-parallel over seeds (4096 -> 512/core on 8 cores); tables replicated.

v3 strategy:
  - Denormalized edge table edet[e] = [emb(64)|pad(64)|feats[u](128)|feats[v](128)]
    bf16 = 768B rows: one descriptor per edge brings everything.
  - Bulk gathers via gpsimd.dma_gather(transpose=True): one instruction per
    (half, bank) window (13 banks of <=32768 rows handle the int16 index
    limit).  The HOST pre-sorts each half's 8192 edge refs by (bank, seed)
    into statically-sized padded windows (pad = repeated valid index, so
    num_idxs is compile-time constant) and ships wrapped int16 index tiles.
  - transpose=True delivers emb^T / fu^T / fv^T directly (no PE transposes).
  - mh / m0 seed-grouping via host-built per-run G matrices [128, 256]
    (1/32 indicators), accumulated in PSUM across each half's blocks.
  - bf16 everywhere; fp32 PSUM accumulate.
"""

import numpy as np

P = 128
NBANK = 13
BANK = 32768
HSEED = 256          # seeds per half
NH = 2               # halves per mp (per core)


# ----------------------------------------------------------------------------
# Host plan: common bank windows, per-(core,mp,half) sorted entries
# ----------------------------------------------------------------------------
def make_plan(inputs, cfg, n_cores):
    S, BC, NMP = cfg["S"], cfg["BC"], cfg["NMP"]
    ids = np.asarray(inputs["ids"]).astype(np.int64)
    n2e = [np.asarray(inputs[f"node2edge_idx_{m}"]).astype(np.int64)
           for m in range(NMP)]

    entries = {}
    counts = np.zeros((NMP, NH, n_cores, NBANK), np.int64)
    for core in range(n_cores):
        for m in range(NMP):
            for h in range(NH):
                seeds = ids[core * BC + h * HSEED: core * BC + (h + 1) * HSEED]
                ef = n2e[m][seeds].ravel()
                s_loc = np.repeat(np.arange(HSEED), S)
                bank = ef >> 15
                loc = (ef & 32767).astype(np.int64)
                order = np.argsort((bank << 13) | s_loc, kind="stable")
                entries[core, m, h] = (bank[order], loc[order], s_loc[order])
                counts[m, h, core] = np.bincount(bank, minlength=NBANK)

    # common static windows: max count over all (m, h, core), rounded to 128
    W = ((counts.reshape(-1, NBANK).max(axis=0) + 127) // 128 * 128)
    W[int(np.argmax(W))] += (-W.sum()) % 512
    TOT = int(W.sum())
    offs = np.concatenate([[0], np.cumsum(W)])[:-1]
    # block -> window map
    blk_bank = np.zeros(TOT // P, np.int64)
    for b in range(NBANK):
        blk_bank[offs[b] // P:(offs[b] + W[b]) // P] = b
    return dict(W=W, TOT=TOT, offs=offs, blk_bank=blk_bank,
                entries=entries, counts=counts)


def build_nc(cfg, plan):
    import concourse.mybir as mybir
    import concourse.bass as bass
    import concourse.tile as tile
    from concourse import bacc

    N, E, S = cfg["N"], cfg["E"], cfg["S"]
    BC, D, DE, NMP = cfg["BC"], cfg["D"], cfg["DE"], cfg["NMP"]
    NCHUNK = BC // P
    EW = 2 * DE + 2 * D      # 384 bf16 elements: emb | pad | fu | fv
    f32 = mybir.dt.float32
    bf16 = mybir.dt.bfloat16
    i16 = mybir.dt.int16
    i32 = mybir.dt.int32

    W, TOT, offs = plan["W"], plan["TOT"], plan["offs"]
    blk_bank = plan["blk_bank"]
    NB = TOT // P
    NG = NB // 4

    nc = bacc.Bacc("TRN2", target_bir_lowering=False)

    feats_bf = nc.dram_tensor("feats_bf", [N, D], bf16, kind="ExternalInput")
    edet = [nc.dram_tensor(f"edet_{m}", [E, EW], bf16, kind="ExternalInput")
            for m in range(NMP)]
    idx_d = nc.dram_tensor("idx", [NMP, NH, P, TOT // 16], i16,
                           kind="ExternalInput")
    g_d = nc.dram_tensor("G", [NMP, NH, P, NB, HSEED], bf16,
                         kind="ExternalInput")
    prep_w = nc.dram_tensor("prep_w", [D, D], bf16, kind="ExternalInput")
    ep_w = nc.dram_tensor("ep_w", [NMP, DE, D], bf16, kind="ExternalInput")
    wn_self = nc.dram_tensor("wn_self", [NMP, 2, D, D], bf16, kind="ExternalInput")
    wn_neigh = nc.dram_tensor("wn_neigh", [NMP, 2, D, D], bf16, kind="ExternalInput")
    we_self = nc.dram_tensor("we_self", [NMP, 2, D, D], bf16, kind="ExternalInput")
    wen0h = nc.dram_tensor("wen0h", [NMP, D, D], bf16, kind="ExternalInput")
    ids_blk = nc.dram_tensor("ids_blk", [P, NCHUNK], i32, kind="ExternalInput")
    ident_d = nc.dram_tensor("ident", [P, P], bf16, kind="ExternalInput")

    out_t = nc.dram_tensor("out", [NMP, BC, 2 * D], f32, kind="ExternalOutput")

    Relu = mybir.ActivationFunctionType.Relu
    IOff = bass.IndirectOffsetOnAxis
    GL = 8   # G blocks staged per load

    with tile.TileContext(nc) as tc:
        with (
            tc.tile_pool(name="wpool", bufs=1) as wp,
            tc.tile_pool(name="gather", bufs=2) as gp,
            tc.tile_pool(name="flat", bufs=1) as fp,
            tc.tile_pool(name="small", bufs=2) as sp,
            tc.tile_pool(name="persist", bufs=1) as pp,
            tc.tile_pool(name="psP", bufs=1, space="PSUM") as psP,
            tc.tile_pool(name="psB", bufs=2, space="PSUM") as psB,
        ):
            def load_w(dram_ap, shape, dtype, tag):
                t = wp.tile(shape, dtype, tag=tag, name=tag)
                nc.sync.dma_start(out=t[:], in_=dram_ap)
                return t

            idsb = load_w(ids_blk[:, :], [P, NCHUNK], i32, "idsb")
            ident = load_w(ident_d[:, :], [P, P], bf16, "ident")
            prepw = load_w(prep_w[:, :], [D, D], bf16, "prepw")
            wns = [[load_w(wn_self[m, l], [D, D], bf16, f"wns_{m}_{l}")
                    for l in range(2)] for m in range(NMP)]
            wnn = [[load_w(wn_neigh[m, l], [D, D], bf16, f"wnn_{m}_{l}")
                    for l in range(2)] for m in range(NMP)]
            wes = [load_w(we_self[m, 0], [D, D], bf16, f"wes_{m}")
                   for m in range(NMP)]
            wenh = [load_w(wen0h[m], [D, D], bf16, f"wenh_{m}")
                    for m in range(NMP)]
            epw = [load_w(ep_w[m], [DE, D], bf16, f"epw_{m}")
                   for m in range(NMP)]

            # ---- seed feats gather (indirect, 4 calls) + x0T
            st = pp.tile([P, NCHUNK, D], bf16, tag="st", name="st")
            for c in range(NCHUNK):
                nc.gpsimd.indirect_dma_start(
                    out=st[:, c, :], out_offset=None, in_=feats_bf[:, :],
                    in_offset=IOff(ap=idsb[:, c:c + 1], axis=0),
                    oob_is_err=False)
            ps_x0 = psB.tile([P, 4 * P], f32, tag="ps_x", name="ps_x0")
            for c in range(NCHUNK):
                nc.tensor.matmul(out=ps_x0[:, c * P:(c + 1) * P],
                                 lhsT=st[:, c, :], rhs=ident[:, :],
                                 start=True, stop=True)
            x0rT = pp.tile([P, BC], bf16, tag="x0rT", name="x0rT")
            nc.vector.tensor_copy(out=x0rT[:, :], in_=ps_x0[:, :])
            ps_x0T = psB.tile([P, 4 * P], f32, tag="ps_x", name="ps_x0T")
            for c in range(NCHUNK):
                nc.tensor.matmul(out=ps_x0T[:, c * P:(c + 1) * P],
                                 lhsT=prepw[:, :], rhs=x0rT[:, c * P:(c + 1) * P],
                                 start=True, stop=True)
            x0T = pp.tile([P, BC], bf16, tag="x0T", name="x0T")
            nc.vector.tensor_copy(out=x0T[:, :], in_=ps_x0T[:, :])

            for m in range(NMP):
                # ---- fold weights
                ps_f = psB.tile([P, 4 * P], f32, tag="ps_h1", name="ps_f")
                nc.tensor.matmul(out=ps_f[0:D, 0:DE], lhsT=epw[m][:, :],
                                 rhs=ident[0:DE, 0:DE], start=True, stop=True)
                epwT = sp.tile([P, DE], bf16, tag="epwT", name="epwT")
                nc.scalar.copy(out=epwT[:, :], in_=ps_f[0:D, 0:DE])

                ps_f2 = psB.tile([P, 4 * P], f32, tag="ps_h1", name="ps_f2")
                nc.tensor.matmul(out=ps_f2[0:DE, 0:P], lhsT=epwT[:, :],
                                 rhs=wes[m][:, :], start=True, stop=True)
                nc.tensor.matmul(out=ps_f2[0:DE, P:2 * P], lhsT=epwT[:, :],
                                 rhs=wnn[m][0][:, :], start=True, stop=True)
                a_t = pp.tile([DE, P], bf16, tag="a_t", name="a_t")
                nc.scalar.copy(out=a_t[:, :], in_=ps_f2[0:DE, 0:P])
                b_t = pp.tile([DE, P], bf16, tag="b_t", name="b_t")
                nc.scalar.copy(out=b_t[:, :], in_=ps_f2[0:DE, P:2 * P])

                ps_f3 = psB.tile([P, 4 * P], f32, tag="ps_h1", name="ps_f3")
                nc.tensor.matmul(out=ps_f3[:, 0:P], lhsT=prepw[:, :],
                                 rhs=ident[:, :], start=True, stop=True)
                prepwT = sp.tile([P, P], bf16, tag="prepwT", name="prepwT")
                nc.scalar.copy(out=prepwT[:, :], in_=ps_f3[:, 0:P])
                ps_f4 = psB.tile([P, 4 * P], f32, tag="ps_h1", name="ps_f4")
                nc.tensor.matmul(out=ps_f4[:, 0:P], lhsT=prepwT[:, :],
                                 rhs=wenh[m][:, :], start=True, stop=True)
                pf_t = pp.tile([P, P], bf16, tag="pf_t", name="pf_t")
                nc.scalar.copy(out=pf_t[:, :], in_=ps_f4[:, 0:P])

                ps_m0 = psP.tile([DE, BC], f32, tag="ps_m0", name="ps_m0")
                ps_mh = psP.tile([P, BC], f32, tag="ps_mh", name="ps_mh")

                for h in range(NH):
                    hc = slice(h * HSEED, (h + 1) * HSEED)

                    idxs = gp.tile([P, TOT // 16], i16, tag="idxs",
                                   name="idxs")
                    nc.sync.dma_start(out=idxs[:, :], in_=idx_d[m, h])

                    # flat dst: window b occupies cols [3*off, 3*(off+wb))
                    ed = gp.tile([P, 3 * TOT], bf16, tag="ed", name="ed")
                    for b in range(NBANK):
                        wb = int(W[b])
                        if wb == 0:
                            continue
                        off = int(offs[b])
                        lo = b * BANK
                        hi = min(E, lo + BANK)
                        dst = ed[:, 3 * off:3 * (off + wb)].rearrange(
                            "p (q w) -> p q w", q=3)
                        nc.gpsimd.dma_gather(
                            dst, edet[m][lo:hi, :],
                            idxs[:, off // 16:(off + wb) // 16],
                            wb, wb, EW, transpose=True, single_packet=False)

                    # xs = fu + fv, per window (flat layout)
                    xs = fp.tile([P, TOT], bf16, tag="xs", name="xs")
                    for b in range(NBANK):
                        wb = int(W[b])
                        if wb == 0:
                            continue
                        o3 = 3 * int(offs[b])
                        nc.vector.tensor_add(
                            out=xs[:, int(offs[b]):int(offs[b]) + wb],
                            in0=ed[:, o3 + wb:o3 + 2 * wb],
                            in1=ed[:, o3 + 2 * wb:o3 + 3 * wb])

                    def emb_slab(j):
                        b = int(blk_bank[j])
                        o3 = 3 * int(offs[b])
                        l = j * P - int(offs[b])
                        return ed[0:DE, o3 + l:o3 + l + P]

                    gt = None
                    for g in range(NG):
                        if g % (GL // 4) == 0:
                            gt = gp.tile([P, GL, HSEED], bf16, tag="gt",
                                         name="gt")
                            j0 = g * 4
                            jn = min(GL, NB - j0)
                            nc.sync.dma_start(
                                out=gt[:, 0:jn, :],
                                in_=g_d[m, h, :, j0:j0 + jn, :])
                        ps_h1 = psB.tile([P, 4 * P], f32, tag="ps_h1",
                                         name="ps_h1")
                        for jj in range(4):
                            j = 4 * g + jj
                            ocs = slice(jj * P, (jj + 1) * P)
                            nc.tensor.matmul(out=ps_h1[:, ocs],
                                             lhsT=emb_slab(j),
                                             rhs=a_t[:, :],
                                             start=True, stop=False)
                            nc.tensor.matmul(out=ps_h1[:, ocs],
                                             lhsT=xs[:, j * P:(j + 1) * P],
                                             rhs=pf_t[:, :],
                                             start=False, stop=True)
                        h1b = sp.tile([P, 4 * P], bf16, tag="h1b", name="h1b")
                        nc.scalar.activation(out=h1b[:, :], in_=ps_h1[:, :],
                                             func=Relu)

                        ps_egr = psB.tile([P, 4 * DE], f32, tag="ps_egr",
                                          name="ps_egr")
                        for jj in range(4):
                            j = 4 * g + jj
                            nc.tensor.matmul(
                                out=ps_egr[:, jj * DE:(jj + 1) * DE],
                                lhsT=emb_slab(j),
                                rhs=ident[0:DE, 0:DE],
                                start=True, stop=True)
                        egr = sp.tile([P, 4 * DE], bf16, tag="egr", name="egr")
                        nc.vector.tensor_copy(out=egr[:, :], in_=ps_egr[:, :])

                        for jj in range(4):
                            j = 4 * g + jj
                            gsl = j % GL
                            first = (g == 0 and jj == 0)
                            last = (g == NG - 1 and jj == 3)
                            nc.tensor.matmul(
                                out=ps_m0[:, hc],
                                lhsT=egr[:, jj * DE:(jj + 1) * DE],
                                rhs=gt[:, gsl, :],
                                start=first, stop=last)
                            nc.tensor.matmul(
                                out=ps_mh[:, hc],
                                lhsT=h1b[:, jj * P:(jj + 1) * P],
                                rhs=gt[:, gsl, :],
                                start=first, stop=last)

                m0b = pp.tile([DE, BC], bf16, tag="m0b", name="m0b")
                nc.scalar.copy(out=m0b[:, :], in_=ps_m0[:, :])
                mhT = pp.tile([P, BC], bf16, tag="mhT", name="mhT")
                nc.vector.tensor_copy(out=mhT[:, :], in_=ps_mh[:, :])

                ps_h0 = psB.tile([P, 4 * P], f32, tag="ps_x", name="ps_h0")
                for c in range(NCHUNK):
                    cs = slice(c * P, (c + 1) * P)
                    nc.tensor.matmul(out=ps_h0[:, cs], lhsT=wns[m][0][:, :],
                                     rhs=x0T[:, cs], start=True, stop=False)
                    nc.tensor.matmul(out=ps_h0[:, cs], lhsT=b_t[:, :],
                                     rhs=m0b[:, cs], start=False, stop=True)
                h0T = pp.tile([P, BC], bf16, tag="h0T", name="h0T")
                nc.scalar.activation(out=h0T[:, :], in_=ps_h0[:, :], func=Relu)

                ps_o1 = psB.tile([P, 4 * P], f32, tag="ps_x", name="ps_o1")
                for c in range(NCHUNK):
                    cs = slice(c * P, (c + 1) * P)
                    nc.tensor.matmul(out=ps_o1[:, cs], lhsT=wns[m][1][:, :],
                                     rhs=h0T[:, cs], start=True, stop=False)
                    nc.tensor.matmul(out=ps_o1[:, cs], lhsT=wnn[m][1][:, :],
                                     rhs=mhT[:, cs], start=False, stop=True)
                o1T = pp.tile([P, BC], bf16, tag="o1T", name="o1T")
                nc.scalar.activation(out=o1T[:, :], in_=ps_o1[:, :], func=Relu)

                for c in range(NCHUNK):
                    cs = slice(c * P, (c + 1) * P)
                    ps_w = psB.tile([P, 4 * P], f32, tag="ps_h1", name="ps_w")
                    nc.tensor.matmul(out=ps_w[:, 0:P], lhsT=h0T[:, cs],
                                     rhs=ident[:, :], start=True, stop=True)
                    nc.tensor.matmul(out=ps_w[:, P:2 * P], lhsT=o1T[:, cs],
                                     rhs=ident[:, :], start=True, stop=True)
                    ob = sp.tile([P, 2 * P], f32, tag="ob", name="ob")
                    nc.vector.tensor_copy(out=ob[:, :], in_=ps_w[:, 0:2 * P])
                    nc.sync.dma_start(
                        out=out_t[m, c * P:(c + 1) * P, :], in_=ob[:, :])

    nc.compile()
    return nc


# ----------------------------------------------------------------------------
# Host-side input preparation
# ----------------------------------------------------------------------------
def make_in_maps(inputs, cfg, plan, n_cores):
    import ml_dtypes
    bf16 = ml_dtypes.bfloat16
    S, BC, NMP, E, DE, D = (cfg["S"], cfg["BC"], cfg["NMP"], cfg["E"],
                            cfg["DE"], cfg["D"])
    NCHUNK = BC // P
    W, TOT, offs, entries = plan["W"], plan["TOT"], plan["offs"], plan["entries"]
    NB = TOT // P

    def bf(x):
        return np.ascontiguousarray(np.asarray(x, dtype=np.float32).astype(bf16))

    ids = np.asarray(inputs["ids"]).astype(np.int32)
    feats = bf(inputs["feats"])

    common = {
        "feats_bf": feats,
        "prep_w": bf(inputs["prep_W"]),
        "ep_w": bf(inputs["edge_prep_W"]),
        "wn_self": bf(inputs["Wn_self"]),
        "wn_neigh": bf(inputs["Wn_neigh"]),
        "we_self": bf(inputs["We_self"]),
        "wen0h": bf(0.5 * np.asarray(inputs["We_neigh"], np.float32)[:, 0]),
        "ident": np.eye(P, dtype=np.float32).astype(bf16),
    }
    zpad = np.zeros((E, DE), bf16)
    for mn in range(NMP):
        adj = np.asarray(inputs[f"edge_node_adj_{mn}"])
        emb = bf(inputs[f"edge_emb_{mn}"])
        common[f"edet_{mn}"] = np.ascontiguousarray(np.concatenate(
            [emb, zpad, feats[adj[:, 0]], feats[adj[:, 1]]], axis=1))

    p_arr = np.arange(P)
    in_maps = []
    for core in range(n_cores):
        shard = ids[core * BC:(core + 1) * BC]
        ids_blk = np.empty((P, NCHUNK), np.int32)
        for c in range(NCHUNK):
            ids_blk[:, c] = shard[c * P + p_arr]
        mm = dict(common)
        mm["ids_blk"] = ids_blk
        idx_all = np.zeros((NMP, NH, P, TOT // 16), np.int16)
        g_all = np.zeros((NMP, NH, P, NB, HSEED), np.float32)
        for m in range(NMP):
            for h in range(NH):
                bank_l, loc_l, s_l = entries[core, m, h]
                idxflat = np.zeros(TOT, np.int16)
                starts = np.searchsorted(bank_l, np.arange(NBANK + 1))
                for b in range(NBANK):
                    lo_i, hi_i = int(starts[b]), int(starts[b + 1])
                    n = hi_i - lo_i
                    if n == 0:
                        continue
                    off = int(offs[b])
                    idxflat[off:off + n] = loc_l[lo_i:hi_i].astype(np.int16)
                    pos = off + np.arange(n)
                    g_all[m, h, pos % P, pos // P, s_l[lo_i:hi_i]] = 1.0 / S
                idxw = idxflat.reshape(TOT // 16, 16).T
                idx_all[m, h] = np.tile(idxw, (8, 1))
        mm["idx"] = np.ascontiguousarray(idx_all)
        mm["G"] = np.ascontiguousarray(g_all.astype(bf16))
        in_maps.append(mm)
    return in_maps


def assemble_output(results, cfg, n_cores):
    NMP, BC, D = cfg["NMP"], cfg["BC"], cfg["D"]
    out = np.empty((NMP, n_cores * BC, 2 * D), np.float32)
    for core in range(n_cores):
        out[:, core * BC:(core + 1) * BC, :] = results[core]["out"]
    return out


FULL_CFG = dict(N=100000, E=400000, S=32, BC=512, D=128, DE=64, NMP=2)


def kernel(**inputs) -> np.ndarray:
    import sys
    for path in ("/opt/trn_rl_repo", "/root/.axon_site/_ro/trn_rl_repo"):
        if path not in sys.path:
            sys.path.append(path)
    from concourse.bass_utils import run_bass_kernel_spmd

    cfg = FULL_CFG
    n_cores = 8
    plan = make_plan(inputs, cfg, n_cores)
    nc = build_nc(cfg, plan)
    in_maps = make_in_maps(inputs, cfg, plan, n_cores)
    res = run_bass_kernel_spmd(nc, in_maps, core_ids=list(range(n_cores)))
    return assemble_output(res.results, cfg, n_cores)
